# revision 1
# baseline (speedup 1.0000x reference)
"""Expert-parallel MoE FFN kernel for Trainium2 (8 NeuronCores).

Strategy: each of the 8 experts lives on its own core. Rows are routed
host-side (argsort by note_type_pos), padded to a uniform capacity C,
and shipped feature-major (transposed) so the device kernel is a pure
dense 2-layer MLP with the feature dimension on SBUF partitions:

    hT = relu(W1.T @ xT + b1)     [F, C]
    yT = W2.T @ hT + b2           [H, C]

Matmuls run in float32r (tf32-like: full fp32 storage, reduced-precision
multiply at full PE rate) with fp32 PSUM accumulation. Weights are
streamed through SBUF once (F blocked into 8 blocks of 512); xT and the
y accumulator stay resident. No collectives needed.
"""

import sys

sys.path.insert(0, "/opt/trn_rl_repo")

import numpy as np

import concourse.bass as bass
import concourse.mybir as mybir
from concourse import bacc
from concourse.tile import TileContext

H = 1024
F = 4096
N_EXPERTS = 8
P = 128
KH = H // P   # 8
KF = F // P   # 32
FB = 512      # F block size (weights streamed per block)
NFB = F // FB  # 8
FC = FB // P  # 4


def _row_tiles(C):
    """Split C columns into equal chunks <=512 (multiples of 16)."""
    n = -(-C // 512)
    rw = -(-C // n)
    rw = ((rw + 15) // 16) * 16
    tiles = []
    s = 0
    while s < C:
        w = min(rw, C - s)
        tiles.append((s, w))
        s += w
    return tiles


def build_expert_kernel(C, reps=1, dt_mm=None):
    """One expert's 2-layer MLP: xT [H, C] -> yT [H, C]."""
    f32 = mybir.dt.float32
    f32r = dt_mm if dt_mm is not None else mybir.dt.float32r
    nc = bacc.Bacc(None, target_bir_lowering=False)
    xT = nc.dram_tensor("xT", [H, C], f32r, kind="ExternalInput")
    w1 = nc.dram_tensor("w1", [H, F], f32r, kind="ExternalInput")
    b1v = nc.dram_tensor("b1v", [P, KF], f32, kind="ExternalInput")
    w2 = nc.dram_tensor("w2", [F, H], f32r, kind="ExternalInput")
    b2v = nc.dram_tensor("b2v", [P, KH], f32, kind="ExternalInput")
    yT = nc.dram_tensor("yT", [H, C], f32, kind="ExternalOutput")

    tiles = _row_tiles(C)
    # xT+yacc residency is 64*C B/partition; drop prefetch depth when a
    # pathological routing pushes C past what bufs=3 pools leave room for.
    wbufs = 3 if C <= 1150 else 2

    with TileContext(nc) as tc:
        with (
            tc.tile_pool(name="consts", bufs=1) as consts,
            tc.tile_pool(name="xp", bufs=1) as xp,
            tc.tile_pool(name="yaccp", bufs=1) as yaccp,
            tc.tile_pool(name="w1p", bufs=wbufs) as w1p,
            tc.tile_pool(name="w2p", bufs=wbufs) as w2p,
            tc.tile_pool(name="hp", bufs=wbufs) as hp,
            tc.tile_pool(name="psh", bufs=4, space="PSUM") as psh,
            tc.tile_pool(name="psy", bufs=4, space="PSUM") as psy,
        ):
            b1_sb = consts.tile([P, KF], f32, tag="b1")
            nc.sync.dma_start(b1_sb[:], b1v[:, :])
            b2_sb = consts.tile([P, KH], f32, tag="b2")
            nc.sync.dma_start(b2_sb[:], b2v[:, :])

            # Issue order tuned for the startup critical path: the first
            # matmul group needs xT[:, :, r0] and w1[fb=0, fc=0] only
            # (~1.9MB), so those DMAs are emitted first.
            xT_sb = xp.tile([P, KH, C], f32r, tag="xT")
            r0_, rw_ = tiles[0]

            def load_w1(fb, fine):
                w1blk = w1p.tile([P, KH, FB], f32r, tag="w1blk")
                if fine:
                    # startup critical path: interleave the (w1, xT) chunk
                    # pairs the first psum group consumes, k-ordered, so no
                    # queue serializes unrelated bytes ahead of them
                    for k in range(KH):
                        nc.sync.dma_start(
                            w1blk[:, k, 0:P],
                            w1[k * P:(k + 1) * P, fb * FB:fb * FB + P])
                        nc.sync.dma_start(xT_sb[:, k, r0_:r0_ + rw_],
                                          xT[k * P:(k + 1) * P, r0_:r0_ + rw_])
                    for fc in range(1, FC):
                        for k in range(KH):
                            nc.sync.dma_start(
                                w1blk[:, k, fc * P:(fc + 1) * P],
                                w1[k * P:(k + 1) * P,
                                   fb * FB + fc * P:fb * FB + (fc + 1) * P])
                else:
                    for k in range(KH):
                        nc.sync.dma_start(
                            w1blk[:, k, :],
                            w1[k * P:(k + 1) * P, fb * FB:(fb + 1) * FB])
                return w1blk

            def load_w2(fb):
                w2blk = w2p.tile([P, FC, H], f32r, tag="w2blk")
                for fc in range(FC):
                    nc.sync.dma_start(
                        w2blk[:, fc, :],
                        w2[fb * FB + fc * P:fb * FB + (fc + 1) * P, :])
                return w2blk

            # PE warmup during the startup DMA window: absorbs the
            # p-state/HAM ramp so real matmuls start at full clock
            wu = consts.tile([P, 512], f32, tag="wu")
            nc.vector.memset(wu[:], 0.0)
            wups = psh.tile([P, 512], f32, tag="ph")
            for i in range(4):
                nc.tensor.matmul(wups[:], wu[:, 0:P], wu[:],
                                 start=(i == 0), stop=(i == 3))

            first_blks = (load_w1(0, fine=True), load_w2(0))
            for (r0, rw) in tiles[1:]:
                for k in range(KH):
                    nc.sync.dma_start(xT_sb[:, k, r0:r0 + rw],
                                      xT[k * P:(k + 1) * P, r0:r0 + rw])

            yacc = yaccp.tile([P, KH, C], f32, tag="yacc")

            def layer1(fb, w1blk, r0, rw):
                h_sb = hp.tile([P, FC, max(t[1] for t in tiles)],
                               f32r, tag="h")
                for fc in range(FC):
                    ph = psh.tile([P, rw], f32, tag="ph")
                    for k in range(KH):
                        nc.tensor.matmul(
                            ph[:],
                            w1blk[:, k, fc * P:(fc + 1) * P],
                            xT_sb[:, k, r0:r0 + rw],
                            start=(k == 0), stop=(k == KH - 1))
                    nc.scalar.activation(
                        h_sb[:, fc, :rw], ph[:],
                        mybir.ActivationFunctionType.Relu,
                        bias=b1_sb[:, fb * FC + fc:fb * FC + fc + 1])
                return h_sb

            def layer2(fb, w2blk, h_sb, m, r0, rw, last):
                py = psy.tile([P, rw], f32, tag="py")
                for fc in range(FC):
                    nc.tensor.matmul(
                        py[:],
                        w2blk[:, fc, m * P:(m + 1) * P],
                        h_sb[:, fc, :rw],
                        start=(fc == 0), stop=(fc == FC - 1))
                if fb == 0:
                    # fold the layer-2 bias into the first partial
                    nc.scalar.activation(
                        yacc[:, m, r0:r0 + rw], py[:],
                        mybir.ActivationFunctionType.Identity,
                        bias=b2_sb[:, m:m + 1])
                else:
                    nc.vector.tensor_add(
                        out=yacc[:, m, r0:r0 + rw],
                        in0=yacc[:, m, r0:r0 + rw], in1=py[:])
                if fb == NFB - 1 and last:
                    # writeback overlaps the remaining compute
                    nc.sync.dma_start(
                        yT[m * P:(m + 1) * P, r0:r0 + rw],
                        yacc[:, m, r0:r0 + rw])

            def body(first_blks=None, last=True):
                for fb in range(NFB):
                    if fb == 0 and first_blks is not None:
                        w1blk, w2blk = first_blks
                    else:
                        w1blk = load_w1(fb, fine=False)
                        w2blk = load_w2(fb)
                    for (r0, rw) in tiles:
                        h_sb = layer1(fb, w1blk, r0, rw)
                        for m in range(KH):
                            layer2(fb, w2blk, h_sb, m, r0, rw, last)

            for i in range(reps - 1):
                body(first_blks if i == 0 else None, last=False)
            body(first_blks if reps == 1 else None, last=True)
    nc.finalize()
    return nc


# SBUF residency (xT + yacc at 64*C bytes/partition) caps per-launch capacity.
MAX_C = 1536


def _prepare(x, note_type_pos, W1, b1, W2, b2, cap):
    """Host-side routing: sort rows by expert, pad to capacity C (<= cap)."""
    ntp = np.asarray(note_type_pos).astype(np.int64)
    x = np.ascontiguousarray(np.asarray(x, dtype=np.float32))
    counts = np.bincount(ntp, minlength=N_EXPERTS)
    C = min(int(counts.max()), cap)
    C = max(16, ((C + 15) // 16) * 16)  # 16-aligned, no extra row-tile padding

    order = np.argsort(ntp, kind="stable")
    weights = []
    for e in range(N_EXPERTS):
        weights.append({
            "w1": np.ascontiguousarray(np.asarray(W1[e], dtype=np.float32)),
            "b1v": np.ascontiguousarray(
                np.asarray(b1[e], dtype=np.float32).reshape(KF, P).T),
            "w2": np.ascontiguousarray(np.asarray(W2[e], dtype=np.float32)),
            "b2v": np.ascontiguousarray(
                np.asarray(b2[e], dtype=np.float32).reshape(KH, P).T),
        })
    # chunk each expert's rows into groups of <= C; one SPMD launch per group
    launches = []
    off = 0
    expert_rows = []
    for e in range(N_EXPERTS):
        expert_rows.append(order[off:off + counts[e]])
        off += counts[e]
    n_launch = max(1, -(-int(counts.max()) // C))
    for g in range(n_launch):
        in_maps, row_idx = [], []
        for e in range(N_EXPERTS):
            rows = expert_rows[e][g * C:(g + 1) * C]
            row_idx.append(rows)
            xe = np.zeros((C, H), dtype=np.float32)
            if len(rows):
                xe[:len(rows)] = x[rows]
            in_maps.append({"xT": np.ascontiguousarray(xe.T), **weights[e]})
        launches.append((in_maps, row_idx))
    return launches, C


def kernel(x, note_type_pos, W1, b1, W2, b2):
    launches, C = _prepare(x, note_type_pos, W1, b1, W2, b2, cap=MAX_C)
    nc = build_expert_kernel(C)
    from concourse.bass_utils import run_bass_kernel_spmd
    T = np.asarray(x).shape[0]
    out = np.zeros((T, H), dtype=np.float32)
    for in_maps, row_idx in launches:
        res = run_bass_kernel_spmd(nc, in_maps, core_ids=list(range(N_EXPERTS)))
        for e in range(N_EXPERTS):
            rows = row_idx[e]
            if len(rows):
                out[rows] = res.results[e]["yT"].T[:len(rows)]
    return out



# revision 13
# speedup vs baseline: 1.1159x; 1.1159x over previous
"""Expert-parallel MoE FFN kernel for Trainium2 (8 NeuronCores).

Strategy: each of the 8 experts lives on its own core. Rows are routed
host-side (argsort by note_type_pos), padded to a uniform capacity C,
and shipped feature-major (transposed) in bf16 so the device kernel is
a pure dense 2-layer MLP with features on SBUF partitions:

    hT = relu(W1.T @ xT + b1)     [F, C]
    yT = W2.T @ hT + b2           [H, C]

In bf16 both expert weight matrices (16MB) fit in SBUF, so weights are
loaded once with a handful of coarse DMAs (the HWDGE charges a fixed
625ns per dma_start, so few/large transfers matter more than bytes) and
stay resident. The row dimension is split into <=512-wide tiles; for
each tile, layer 1 runs 32 fc-groups (8 k-matmuls PSUM-accumulated,
then a fused relu+bias to bf16 h), and layer 2 runs 8 m-groups that
accumulate all 32 fc contributions in a single PSUM bank before one
fused bias+identity drains to f32 and DMAs out. No cross-fb SBUF
accumulation exists, so the vector engine is idle and the drain after
the last matmul is just one activation + one small DMA.
"""

import sys

sys.path.insert(0, "/opt/trn_rl_repo")

import numpy as np

import concourse.bass as bass
import concourse.mybir as mybir
from concourse import bacc
from concourse.tile import TileContext

H = 1024
F = 4096
N_EXPERTS = 8
P = 128
KH = H // P    # 8   (H partition blocks = layer-2 output blocks m)
KF = F // P    # 32  (F partition blocks = layer-1 output groups fc)
G1 = 512       # W1 DMA group width (f columns per load)
NG1 = F // G1  # 8

NP_BF16 = mybir.dt.np(mybir.dt.bfloat16)


def _row_tiles(C):
    """Split C columns into equal chunks <=512 (multiples of 16)."""
    n = -(-C // 512)
    rw = -(-C // n)
    rw = ((rw + 15) // 16) * 16
    tiles = []
    s = 0
    while s < C:
        w = min(rw, C - s)
        tiles.append((s, w))
        s += w
    return tiles


def build_expert_kernel(C, reps=1):
    """One expert's 2-layer MLP: xT [H, C] -> yT [H, C], weights resident."""
    f32 = mybir.dt.float32
    bf16 = mybir.dt.bfloat16
    nc = bacc.Bacc(None, target_bir_lowering=False)
    # Host-packed DRAM layouts, all chosen so every DMA's innermost
    # contiguous run is >=512B (full DMA-engine rate):
    #   xT  [128p, 8k, C]      xT[p,k,c]   = x[c, k*128+p]
    #   w1  [128p, 8k, F]      w1[p,k,f]   = W1[k*128+p, f]
    #   w2  [8m, 128p, 32fc, 128]  w2[m,p,fc,c] = W2[fc*128+p, m*128+c]
    xT = nc.dram_tensor("xT", [P, KH, C], bf16, kind="ExternalInput")
    w1 = nc.dram_tensor("w1", [P, KH, F], bf16, kind="ExternalInput")
    w2 = nc.dram_tensor("w2", [KH, P, KF, P], bf16, kind="ExternalInput")
    b1v = nc.dram_tensor("b1v", [P, KF], f32, kind="ExternalInput")
    b2v = nc.dram_tensor("b2v", [P, KH], f32, kind="ExternalInput")
    yT = nc.dram_tensor("yT", [H, C], bf16, kind="ExternalOutput")

    tiles = _row_tiles(C)
    wmax = max(t[1] for t in tiles)

    with TileContext(nc) as tc:
        with (
            tc.tile_pool(name="consts", bufs=1) as consts,
            tc.tile_pool(name="xp", bufs=1) as xp,
            tc.tile_pool(name="w1p", bufs=1) as w1p,
            tc.tile_pool(name="w2p", bufs=1) as w2p,
            tc.tile_pool(name="hp", bufs=1) as hp,
            tc.tile_pool(name="yst", bufs=4) as yst,
            tc.tile_pool(name="psh", bufs=3, space="PSUM") as psh,
            tc.tile_pool(name="psy", bufs=4, space="PSUM") as psy,
        ):
            b1_sb = consts.tile([P, KF], f32, tag="b1")
            b2_sb = consts.tile([P, KH], f32, tag="b2")
            x_sb = xp.tile([P, KH, C], bf16, tag="xT")
            w1_sb = w1p.tile([P, KH, F], bf16, tag="w1")
            w2_sb = w2p.tile([P, KH, KF, P], bf16, tag="w2")

            # Startup-critical DMA order (HWDGE charges 625ns per issue and
            # DMA_ENGINES transfers strictly in order): W1's first fc block
            # and x[tile0] (split so the first k-matmuls start on the first
            # half) lead; W1 streams in groups sized to stay just ahead of
            # the fc-group consumption cadence; W2 follows with b2 ahead of
            # the first layer-2 activation.
            r0_, rw_ = tiles[0]

            def w1_load(c0, c1):
                nc.sync.dma_start(w1_sb[:, :, c0:c1], w1[:, :, c0:c1])

            w1_load(0, 2 * P)
            nc.sync.dma_start(x_sb[:, 0:KH // 2, r0_:r0_ + rw_],
                              xT[:, 0:KH // 2, r0_:r0_ + rw_])
            nc.sync.dma_start(x_sb[:, KH // 2:KH, r0_:r0_ + rw_],
                              xT[:, KH // 2:KH, r0_:r0_ + rw_])
            nc.sync.dma_start(b1_sb[:], b1v[:, :])
            for q in range(1, KF // 2):    # 256-col groups: cols 256..4096
                w1_load(q * 2 * P, (q + 1) * 2 * P)
            if len(tiles) > 1:
                nc.sync.dma_start(x_sb[:, :, rw_:C], xT[:, :, rw_:C])
            nc.sync.dma_start(b2_sb[:], b2v[:, :])
            for m in range(KH):
                nc.sync.dma_start(w2_sb[:, m, :, :], w2[m, :, :, :])

            # PE warmup: absorbs the p-state ramp during the startup DMA
            # window so real matmuls start at full clock; sized to end just
            # as the first fc-group's data lands.
            wu = consts.tile([P, 512], f32, tag="wu")
            nc.vector.memset(wu[:], 0.0)
            wups = psh.tile([P, 512], f32, tag="ph")
            for i in range(2):
                nc.tensor.matmul(wups[:, 0:416], wu[:, 0:P], wu[:, 0:416],
                                 start=(i == 0), stop=(i == 1))

            h_sb = hp.tile([P, KF, wmax], bf16, tag="h")

            def layer1(r0, rw):
                for fc in range(KF):
                    ph = psh.tile([P, rw], f32, tag="ph")
                    for k in range(KH):
                        nc.tensor.matmul(
                            ph[:],
                            w1_sb[:, k, fc * P:(fc + 1) * P],
                            x_sb[:, k, r0:r0 + rw],
                            start=(k == 0), stop=(k == KH - 1))
                    nc.scalar.activation(
                        h_sb[:, fc, :rw], ph[:],
                        mybir.ActivationFunctionType.Relu,
                        bias=b1_sb[:, fc:fc + 1])

            def l2_group(m, r0, l0, lw, last):
                # r0: tile's global column offset (for yT); l0: local column
                # offset within the tile's h slab.
                py = psy.tile([P, lw], f32, tag="py")
                for fc in range(KF):
                    nc.tensor.matmul(
                        py[:],
                        w2_sb[:, m, fc, :],
                        h_sb[:, fc, l0:l0 + lw],
                        start=(fc == 0), stop=(fc == KF - 1))
                yo = yst.tile([P, wmax], bf16, tag="yo")
                nc.scalar.activation(
                    yo[:, :lw], py[:],
                    mybir.ActivationFunctionType.Identity,
                    bias=b2_sb[:, m:m + 1])
                if last:
                    nc.sync.dma_start(
                        yT[m * P:(m + 1) * P, r0 + l0:r0 + l0 + lw],
                        yo[:, :lw])

            def layer2(r0, rw, last, final):
                for m in range(KH):
                    if final and m == KH - 1:
                        # Split the kernel's very last m-group so the tail
                        # act->DMA->sem chain drains only a 48-col chunk.
                        hw_ = rw - 48 if rw > 64 else rw // 2
                        l2_group(m, r0, 0, hw_, last)
                        l2_group(m, r0, hw_, rw - hw_, last)
                    else:
                        l2_group(m, r0, 0, rw, last)

            for rep in range(reps):
                last = rep == reps - 1
                for ti, (r0, rw) in enumerate(tiles):
                    layer1(r0, rw)
                    layer2(r0, rw, last, last and ti == len(tiles) - 1)
    nc.finalize()
    return nc


# SBUF residency: weights 128KB/partition + x (2*KH*C B) + h (2*KF*512 B).
MAX_C = 1536


def _prepare(x, note_type_pos, W1, b1, W2, b2, cap):
    """Host-side routing: sort rows by expert, pad to capacity C (<= cap),
    pack per-expert tensors into the kernel's DMA-friendly layouts."""
    ntp = np.asarray(note_type_pos).astype(np.int64)
    x = np.ascontiguousarray(np.asarray(x, dtype=np.float32))
    counts = np.bincount(ntp, minlength=N_EXPERTS)
    C = min(int(counts.max()), cap)
    C = max(16, ((C + 15) // 16) * 16)

    order = np.argsort(ntp, kind="stable")
    weights = []
    for e in range(N_EXPERTS):
        w1e = np.asarray(W1[e], dtype=np.float32).astype(NP_BF16)
        w2e = np.asarray(W2[e], dtype=np.float32).astype(NP_BF16)
        weights.append({
            # [128p, 8k, F]
            "w1": np.ascontiguousarray(
                w1e.reshape(KH, P, F).transpose(1, 0, 2)),
            # [8m, 128p, 32fc, 128c]
            "w2": np.ascontiguousarray(
                w2e.reshape(KF, P, KH, P).transpose(2, 1, 0, 3)),
            "b1v": np.ascontiguousarray(
                np.asarray(b1[e], dtype=np.float32).reshape(KF, P).T),
            "b2v": np.ascontiguousarray(
                np.asarray(b2[e], dtype=np.float32).reshape(KH, P).T),
        })
    launches = []
    off = 0
    expert_rows = []
    for e in range(N_EXPERTS):
        expert_rows.append(order[off:off + counts[e]])
        off += counts[e]
    n_launch = max(1, -(-int(counts.max()) // C))
    for g in range(n_launch):
        in_maps, row_idx = [], []
        for e in range(N_EXPERTS):
            rows = expert_rows[e][g * C:(g + 1) * C]
            row_idx.append(rows)
            xe = np.zeros((C, H), dtype=np.float32)
            if len(rows):
                xe[:len(rows)] = x[rows]
            # [128p, 8k, C]
            xpack = np.ascontiguousarray(
                xe.T.astype(NP_BF16).reshape(KH, P, C).transpose(1, 0, 2))
            in_maps.append({"xT": xpack, **weights[e]})
        launches.append((in_maps, row_idx))
    return launches, C


def kernel(x, note_type_pos, W1, b1, W2, b2):
    launches, C = _prepare(x, note_type_pos, W1, b1, W2, b2, cap=MAX_C)
    nc = build_expert_kernel(C)
    from concourse.bass_utils import run_bass_kernel_spmd
    T = np.asarray(x).shape[0]
    out = np.zeros((T, H), dtype=np.float32)
    for in_maps, row_idx in launches:
        res = run_bass_kernel_spmd(nc, in_maps, core_ids=list(range(N_EXPERTS)))
        for e in range(N_EXPERTS):
            rows = row_idx[e]
            if len(rows):
                out[rows] = res.results[e]["yT"].astype(np.float32).T[:len(rows)]
    return out
